# revision 35
# baseline (speedup 1.0000x reference)
"""Trainium2 Bass kernel for MultiHeadMemAttn (mean-pooled-memory attention).

Full computation (per batch b):
    mem  = mean_pool(keyvalue, window=64, stride=64)          # [64, 512]
    hq   = query @ Wq.T ; hk = mem @ Wk.T ; hv = mem @ Wv.T   # heads=8, hd=64
    attn = softmax(hq @ hk.T / 8, over mem axis)
    out  = (attn @ hv) @ Wo.T

Sharding: data-parallel over batch across 8 cores (4 batches each),
weights replicated.  No collectives.

Device strategy (per core):
  - all matmul operands use float32r (4-byte storage, reduced-precision PE
    path): 1 cycle/row instead of fp32's 4 cycles/row, ~7e-5 rel err.
  - pooling is a PE matmul against slices of a host-built band matrix,
    accumulated over 32 s-chunks in PSUM.
  - query tiles are PE-transposed to qT [d, i]; hqT = WqT-chunks.T @ qT.
  - scores computed transposed (scoresT[j, i] = hkT_h.T @ hqT_h), head
    pairs packed into one [128, 512] PSUM tile; softmax without
    max-subtraction (|scores| <= ~6): E = exp(s/8) on ACT, denominators via
    a K=128 ones-matmul -> [2,512], reciprocal_approx on DVE, partition
    broadcast on (idle) GPSIMD, normalize on DVE.
  - uvecT[dv, i] = V_h-as-lhsT.T @ attn (vecT comes out pre-transposed).
  - out[i, o] = vecT-chunks.T @ WoT in natural layout, DMA'd straight out.
"""

import os
from contextlib import ExitStack

import numpy as np

import concourse.bass as bass
import concourse.mybir as mybir
import concourse.tile as tile
from concourse.bass_utils import run_bass_kernel_spmd

F32 = mybir.dt.float32
F32R = mybir.dt.float32r

NCORES = 8
B = 4          # batches per core
QLEN = 1024
S = 4096       # kv sequence length
D = 512        # hidden
H = 8          # heads
HD = 64        # head dim
MEM = 64       # mem_len (pooled length)
DC = D // 128  # 4 chunks of the hidden dim
ICN = 2        # i-chunks of 512 per batch
IT = 4         # 128-row tiles per i-chunk
KT = 8         # kv DMA tiles per batch (4 s-chunks of 128 each)

EXPF = mybir.ActivationFunctionType.Exp

# ---------------------------------------------------------------------------
# Workaround: this walrus build only encodes ONE sem-wait per instruction
# ("Too many sync wait commands" in CoreV3GenImpl setupSyncWait), while
# Tile's sem-assignment freely attaches several.  Post-process the
# serialized BIR: move surplus waits onto injected same-engine NoOps placed
# immediately before the instruction (engine streams are in-order, so the
# NoOp chain stalls the engine exactly like multi-wait would).
import json as _json

_orig_to_json_bytes = bass.Bass.to_json_bytes


def _split_multi_waits(self, *args, **kwargs):
    raw = _orig_to_json_bytes(self, *args, **kwargs)
    d = _json.loads(raw)
    changed = False

    def fix_block(o):
        nonlocal changed
        if isinstance(o, dict):
            insts = o.get("instructions")
            if isinstance(insts, list):
                new = []
                for inst in insts:
                    si = inst.get("sync_info") if isinstance(inst, dict) else None
                    waits = (si or {}).get("on_wait") or []
                    if len(waits) > 1:
                        changed = True
                        for i, w in enumerate(waits[:-1]):
                            new.append(
                                {
                                    "name": f"{inst['name']}-sw{i}",
                                    "opcode": "NoOp",
                                    "engine": inst["engine"],
                                    "ins": [],
                                    "outs": [],
                                    "debug": inst.get("debug", 0),
                                    "sync_info": {
                                        "on_wait": [w],
                                        "on_update": [],
                                    },
                                }
                            )
                        si["on_wait"] = [waits[-1]]
                    new.append(inst)
                o["instructions"] = new
            for v in o.values():
                fix_block(v)
        elif isinstance(o, list):
            for v in o:
                fix_block(v)

    fix_block(d)
    if not changed:
        return raw
    return _json.dumps(d).encode()


bass.Bass.to_json_bytes = _split_multi_waits
# ---------------------------------------------------------------------------


def _build_nc() -> bass.Bass:
    nc = bass.Bass()
    # inputs declared float32r: same 4-byte layout as the f32 numpy arrays
    # we feed in; the PE reads them at reduced precision / full speed.
    q = nc.dram_tensor("queryT", [B, D, QLEN], F32R, kind="ExternalInput")
    kv = nc.dram_tensor("keyvalue", [B, S, D], F32R, kind="ExternalInput")
    wqT = nc.dram_tensor("wqT", [D, D], F32R, kind="ExternalInput")
    wkT = nc.dram_tensor("wkT", [D, D], F32R, kind="ExternalInput")
    wvT = nc.dram_tensor("wvT", [D, D], F32R, kind="ExternalInput")
    woT = nc.dram_tensor("woT", [D, D], F32R, kind="ExternalInput")
    poolD = nc.dram_tensor("poolD", [128, 126], F32R, kind="ExternalInput")
    ident = nc.dram_tensor("ident", [128, 128], F32R, kind="ExternalInput")
    ones2 = nc.dram_tensor("ones2", [128, 4, 32], F32R, kind="ExternalInput")
    expand2 = nc.dram_tensor("expand2", [32, 4, 128], F32R, kind="ExternalInput")
    out = nc.dram_tensor("out", [B, QLEN, D], F32, kind="ExternalOutput")

    # DRAM views for partition-major DMA
    q_v = q.rearrange("b (dc p) (ic i) -> b ic p dc i", p=128, ic=ICN)
    kv_v = kv.rearrange("b (t c p) d -> b t p c d", t=KT, c=4, p=128)
    out_v = out.rearrange("b (ic it p) d -> b ic p it d", ic=ICN, it=IT, p=128)
    wq_v = wqT.rearrange("(dc p) o -> p dc o", p=128)
    wk_v = wkT.rearrange("(dc p) o -> p dc o", p=128)
    wv_v = wvT.rearrange("(dc p) o -> p dc o", p=128)
    wo_v = woT.rearrange("(dc p) o -> p dc o", p=128)

    with tile.TileContext(nc) as tc, ExitStack() as ctx:
        # SBUF pools
        singles = ctx.enter_context(tc.tile_pool(name="singles", bufs=1))
        kvp = ctx.enter_context(tc.tile_pool(name="kvp", bufs=5))
        qtp = ctx.enter_context(tc.tile_pool(name="qtp", bufs=2))
        hqp = ctx.enter_context(tc.tile_pool(name="hqp", bufs=3))
        memp = ctx.enter_context(tc.tile_pool(name="memp", bufs=2))
        ep = ctx.enter_context(tc.tile_pool(name="ep", bufs=9))
        attnp = ctx.enter_context(tc.tile_pool(name="attnp", bufs=4))
        rdp = ctx.enter_context(tc.tile_pool(name="rdp", bufs=3))
        bcp = ctx.enter_context(tc.tile_pool(name="bcp", bufs=3))
        vtp = ctx.enter_context(tc.tile_pool(name="vtp", bufs=3))
        outp = ctx.enter_context(tc.tile_pool(name="outp", bufs=2))
        # PSUM pools (8 banks total: 1 + 1 + 6)
        accp = ctx.enter_context(tc.tile_pool(name="accp", bufs=1, space="PSUM"))
        denp = ctx.enter_context(tc.tile_pool(name="denp", bufs=1, space="PSUM"))
        mmp = ctx.enter_context(tc.tile_pool(name="mmp", bufs=6, space="PSUM"))

        # one-time loads: tiny consts, Wq, then batch-0 kv (so pooling can
        # start quickly), then the remaining weight matrices.
        poolD_sb = singles.tile([128, 126], F32R)
        nc.sync.dma_start(out=poolD_sb, in_=poolD[:, :])
        ident_sb = singles.tile([128, 128], F32R)
        nc.sync.dma_start(out=ident_sb, in_=ident[:, :])
        ones2_sb = singles.tile([128, 4, 32], F32R)
        nc.sync.dma_start(out=ones2_sb, in_=ones2[:, :, :])
        expand2_sb = singles.tile([32, 4, 128], F32R)
        nc.sync.dma_start(out=expand2_sb, in_=expand2[:, :, :])
        wq_sb = singles.tile([128, DC, D], F32R)
        nc.sync.dma_start(out=wq_sb, in_=wq_v)
        kv0_tiles = {}
        qT0_pre = {}

        def _kv0(t):
            kvt = kvp.tile([128, 4, D], F32R, tag="kv", name=f"kv0_{t}")
            nc.sync.dma_start(out=kvt, in_=kv_v[0, t])
            kv0_tiles[t] = kvt

        _kv0(0)
        _kv0(1)
        qT00 = qtp.tile([128, DC, D], F32R, tag="qT", name="qT00")
        nc.sync.dma_start(out=qT00, in_=q_v[0, 0])
        qT0_pre[0] = qT00
        _kv0(2)
        _kv0(3)
        qT01 = qtp.tile([128, DC, D], F32R, tag="qT", name="qT01")
        nc.sync.dma_start(out=qT01, in_=q_v[0, 1])
        qT0_pre[1] = qT01
        wk_sb = singles.tile([128, DC, D], F32R)
        nc.sync.dma_start(out=wk_sb, in_=wk_v)
        wv_sb = singles.tile([128, DC, D], F32R)
        nc.sync.dma_start(out=wv_sb, in_=wv_v)
        _kv0(4)
        _kv0(5)
        wo_sb = singles.tile([128, DC, D], F32R)
        nc.sync.dma_start(out=wo_sb, in_=wo_v)

        def make_prep(b):
            """Returns (chunk_steps, tail_fn, state). Each chunk step loads
            one kv tile and runs its pool matmuls; steps are interleaved
            into other units' emission so the DMA-paced pool never stalls
            the in-order PE queue."""
            st = {}
            pacc = accp.tile([MEM, D], F32, tag="acc")

            def chunk_step(t):
                def f():
                    if b == 0 and t in kv0_tiles:
                        kvt = kv0_tiles[t]
                    else:
                        kvt = kvp.tile([128, 4, D], F32R, tag="kv")
                        nc.sync.dma_start(out=kvt, in_=kv_v[b, t])
                    for c in range(4):
                        sc = 4 * t + c
                        nc.tensor.matmul(
                            pacc,
                            lhsT=poolD_sb[:, 62 - 2 * sc : 126 - 2 * sc],
                            rhs=kvt[:, c, :],
                            start=(sc == 0),
                            stop=(sc == 31),
                        )
                return f

            def tail():
                mem_sb = memp.tile([MEM, D], F32R, tag="mem")
                nc.scalar.copy(out=mem_sb, in_=pacc)
                trt = mmp.tile([128, 4, MEM], F32R, tag="mm")
                for c in range(4):
                    nc.tensor.transpose(
                        trt[:, c, :],
                        mem_sb[:, 128 * c : 128 * (c + 1)],
                        ident_sb[0:MEM, 0:MEM],
                    )
                memT_sb = memp.tile([128, 4, MEM], F32R, tag="memT")
                nc.scalar.copy(out=memT_sb, in_=trt)
                hkbd_sb = memp.tile([128, 4, 128], F32R, tag="hkbd")
                nc.scalar.mul(out=hkbd_sb, in_=wk_sb[:, 0, :], mul=0.0)
                for oc in range(4):
                    hk_ps = mmp.tile([128, MEM], F32, tag="mm")
                    for dc in range(DC):
                        nc.tensor.matmul(
                            hk_ps,
                            lhsT=wk_sb[:, dc, 128 * oc : 128 * (oc + 1)],
                            rhs=memT_sb[:, dc, :],
                            start=(dc == 0),
                            stop=(dc == DC - 1),
                        )
                    nc.scalar.mul(
                        out=hkbd_sb[0:64, oc, 0:64], in_=hk_ps[0:64, :], mul=0.125
                    )
                    nc.scalar.mul(
                        out=hkbd_sb[64:128, oc, 64:128],
                        in_=hk_ps[64:128, :],
                        mul=0.125,
                    )
                hv_ps = mmp.tile([MEM, D], F32, tag="mm")
                for dc in range(DC):
                    nc.tensor.matmul(
                        hv_ps,
                        lhsT=memT_sb[:, dc, :],
                        rhs=wv_sb[:, dc, :],
                        start=(dc == 0),
                        stop=(dc == DC - 1),
                    )
                hv_sb = memp.tile([MEM, D], F32R, tag="hv")
                nc.scalar.copy(out=hv_sb, in_=hv_ps)
                hvbd_sb = memp.tile([128, 4, 128], F32R, tag="hvbd")
                nc.scalar.mul(out=hvbd_sb, in_=wv_sb[:, 0, :], mul=0.0)
                ev = hv_sb.rearrange("m (p2 two dv) -> m p2 two dv", p2=4, two=2)
                nc.scalar.copy(out=hvbd_sb[0:64, :, 0:64], in_=ev[:, :, 0, :])
                nc.sync.dma_start(
                    out=hvbd_sb[64:128, :, 64:128], in_=ev[:, :, 1, :]
                )
                st["hkbd"] = hkbd_sb
                st["hvbd"] = hvbd_sb

            return [chunk_step(t) for t in range(KT)], tail, st

        def emit_A(bst, b, ic, qT_pre=None):
            """qT load, hq, scores, exp, denominators, reciprocal."""
            st = {"b": b, "ic": ic, "bst": bst}
            if qT_pre is not None:
                qT_sb = qT_pre
            else:
                qT_sb = qtp.tile([128, DC, D], F32R, tag="qT")
                nc.sync.dma_start(out=qT_sb, in_=q_v[b, ic])
            hqT_sb = hqp.tile([128, DC, D], F32R, tag="hqT")
            for oc in range(DC):
                hq_ps = mmp.tile([128, D], F32, tag="mm")
                for dc in range(DC):
                    nc.tensor.matmul(
                        hq_ps,
                        lhsT=wq_sb[:, dc, 128 * oc : 128 * (oc + 1)],
                        rhs=qT_sb[:, dc, :],
                        start=(dc == 0),
                        stop=(dc == DC - 1),
                    )
                nc.scalar.copy(out=hqT_sb[:, oc, :], in_=hq_ps)
            den_ps = denp.tile([32, D], F32, tag="den")
            e_tiles = []
            for p2 in range(4):
                sc_ps = mmp.tile([128, D], F32, tag="mm")
                nc.tensor.matmul(
                    sc_ps,
                    lhsT=bst["hkbd"][:, p2, :],
                    rhs=hqT_sb[:, p2, :],
                    start=True,
                    stop=True,
                )
                e_sb = ep.tile([128, D], F32R, tag="e")
                nc.scalar.activation(out=e_sb, in_=sc_ps, func=EXPF)
                e_tiles.append(e_sb)
                nc.tensor.matmul(
                    den_ps,
                    lhsT=ones2_sb[:, p2, :],
                    rhs=e_sb,
                    start=(p2 == 0),
                    stop=(p2 == 3),
                )
            rden_r = rdp.tile([32, D], F32R, tag="rden")
            with nc.allow_low_precision(reason="f32r reciprocal feeds f32r matmul"):
                nc.vector.reciprocal(out=rden_r, in_=den_ps)
            st["e"] = e_tiles
            st["rden"] = rden_r
            return st

        def emit_B(st, filler, split_store=False):
            """normalize, V-matmul, Wo projection, store for unit st.
            After each head pair, emit one pending pool-chunk of the next
            batch (keeps the PE queue fed while this unit's DVE/ACT run)."""
            b, ic, bst = st["b"], st["ic"], st["bst"]
            vecT_sb = vtp.tile([128, 4, D], F32R, tag="vecT")
            for p2 in range(4):
                if filler:
                    filler.pop(0)()
                bc_ps = mmp.tile([128, D], F32, tag="mm")
                nc.tensor.matmul(
                    bc_ps,
                    lhsT=expand2_sb[:, p2, :],
                    rhs=st["rden"],
                    start=True,
                    stop=True,
                )
                attn_sb = attnp.tile([128, D], F32R, tag="attn")
                nc.vector.tensor_mul(attn_sb, st["e"][p2], bc_ps)
                uv_ps = mmp.tile([128, D], F32, tag="mm")
                nc.tensor.matmul(
                    uv_ps,
                    lhsT=bst["hvbd"][:, p2, :],
                    rhs=attn_sb,
                    start=True,
                    stop=True,
                )
                if p2 < 2:
                    nc.scalar.copy(out=vecT_sb[:, p2, :], in_=uv_ps)
                else:
                    nc.vector.tensor_copy(out=vecT_sb[:, p2, :], in_=uv_ps)
            out_sb = outp.tile([128, IT, D], F32, tag="o")
            for it in range(IT):
                o_ps = mmp.tile([128, D], F32, tag="mm")
                for hc in range(4):
                    nc.tensor.matmul(
                        o_ps,
                        lhsT=vecT_sb[:, hc, 128 * it : 128 * (it + 1)],
                        rhs=wo_sb[:, hc, :],
                        start=(hc == 0),
                        stop=(hc == 3),
                    )
                nc.vector.tensor_copy(out=out_sb[:, it, :], in_=o_ps)
                if split_store:
                    nc.scalar.dma_start(
                        out=out_v[b, ic][:, it, :], in_=out_sb[:, it, :]
                    )
            if not split_store:
                # store on the second HWDGE ring (ACT) so loads/stores overlap
                nc.scalar.dma_start(out=out_v[b, ic], in_=out_sb)

        # software pipeline: A(k+1) is emitted before B(k) so the PE has
        # matmul work while unit k's reciprocal runs; the next batch's
        # DMA-paced pool chunks are woven into B phases.
        units = [(b, ic) for b in range(B) for ic in range(ICN)]
        steps0, tail0, st0 = make_prep(0)
        for f in steps0:
            f()
        tail0()
        batch_state = {0: st0}
        for b in range(B):
            pre = qT0_pre if b == 0 else {}
            stA0 = emit_A(batch_state[b], b, 0, qT_pre=pre.get(0))
            stA1 = emit_A(batch_state[b], b, 1, qT_pre=pre.get(1))
            if b + 1 < B:
                next_steps, next_tail, next_st = make_prep(b + 1)
            else:
                next_steps, next_tail, next_st = [], None, None
            emit_B(stA0, next_steps)
            emit_B(stA1, next_steps, split_store=(b == B - 1))
            for f in next_steps:
                f()
            if next_tail is not None:
                next_tail()
                batch_state[b + 1] = next_st
    return nc


_NC = None


def _get_nc() -> bass.Bass:
    global _NC
    if _NC is None:
        _NC = _build_nc()
    return _NC


def _consts() -> dict:
    poolD = np.zeros((128, 126), np.float32)
    poolD[0:64, 62] = 1.0 / 64.0
    poolD[64:128, 63] = 1.0 / 64.0
    ident = np.eye(128, dtype=np.float32)
    # den matmul lhsT (per pair p2): accumulate into one [32, D] tile; row
    # 2*p2 sums even-head exp rows (partitions 0-63), row 2*p2+1 sums odd
    # (64-127); rows 8-31 duplicate row 0's pattern so every PSUM row gets
    # a finite positive value (reciprocal runs on the whole tile).
    ones2 = np.zeros((128, 4, 32), np.float32)
    for p2 in range(4):
        ones2[0:64, p2, 2 * p2] = 1.0
        ones2[64:128, p2, 2 * p2 + 1] = 1.0
        if p2 == 0:
            ones2[0:64, p2, 8:32] = 1.0
    # broadcast matmul lhsT, one [32, 128] slice per head pair
    expand2 = np.zeros((32, 4, 128), np.float32)
    for p2 in range(4):
        expand2[2 * p2 + 0, p2, 0:64] = 1.0
        expand2[2 * p2 + 1, p2, 64:128] = 1.0
    return dict(poolD=poolD, ident=ident, ones2=ones2, expand2=expand2)


def run(inputs: dict, trace: bool = False):
    """Run on 8 cores; returns (full_output, BassKernelResults)."""
    query = np.asarray(inputs["query"], np.float32)
    queryT = np.ascontiguousarray(query.transpose(0, 2, 1))
    keyvalue = np.ascontiguousarray(np.asarray(inputs["keyvalue"], np.float32))
    w = {
        "wqT": np.ascontiguousarray(np.asarray(inputs["Wq"], np.float32).T),
        "wkT": np.ascontiguousarray(np.asarray(inputs["Wk"], np.float32).T),
        "wvT": np.ascontiguousarray(np.asarray(inputs["Wv"], np.float32).T),
        "woT": np.ascontiguousarray(np.asarray(inputs["Wo"], np.float32).T),
    }
    consts = _consts()
    nb = query.shape[0]
    per = nb // NCORES
    assert per == B, f"expected {NCORES * B} batches, got {nb}"

    in_maps = []
    for k in range(NCORES):
        m = {
            "queryT": np.ascontiguousarray(queryT[k * per : (k + 1) * per]),
            "keyvalue": np.ascontiguousarray(keyvalue[k * per : (k + 1) * per]),
        }
        m.update(w)
        m.update(consts)
        in_maps.append(m)

    res = run_bass_kernel_spmd(
        _get_nc(), in_maps, core_ids=list(range(NCORES)), trace=trace
    )
    outs = [r["out"] for r in res.results]
    return np.concatenate(outs, axis=0), res


def kernel(**inputs) -> np.ndarray:
    out, _ = run(inputs, trace=False)
    return out
